# revision 36
# baseline (speedup 1.0000x reference)
"""Trainium2 Bass kernel for a pre-norm transformer block with dilated
windowed causal attention (B=2, L=2048, D=512, H=8, DIL=2, WIN=256,
HIDDEN=2048).

Sharding: 8 cores = batch(2) x sequence-chunk(4 x 512 tokens). Each core
receives its 512-token chunk plus a 256-token halo (keys/values only) and
computes the full block for its tokens; no collectives.

v6 notes (on top of v3..v5):
  All projection weights ship fp8 (qkv/out scaled x64 host-side, FFN
  x2048); xT/oT are fp8 so QKV, V, out-proj and FFN all run DoubleRow
  fp8 matmuls at 2x.  Both layer-norm applies are folded into the
  transpose matmuls (diag(rstd) moving operand + one batched rank-1 mean
  mm per psum bank).  LN2 stats read the out-proj PSUM directly.  QKV is
  emitted per-head-pair interleaved with attention chains; junk-matmul
  fillers bridge the two PE gaps so the HAM clock gate stays warm.  ACT
  table prefetch dummies are data-dependent so the scheduler cannot
  hoist them.
"""
import os
import sys

os.environ.setdefault("MYCRO_LOCAL_CACHE", "1")
if "/opt/trn_rl_repo" not in sys.path:
    sys.path.insert(0, "/opt/trn_rl_repo")

import numpy as np

B, L, D, H, HD = 2, 2048, 512, 8, 64
HIDDEN = 4 * D
P = 128
CH = 512            # own tokens per core
HALO = 256
T = CH + HALO       # 768
NCORES = 8
EPS = 1e-5
SL = T // 2         # 384 keys per parity stream
SQ = CH // 2        # 256 queries per parity stream
SW = 128            # causal window in stream coords
SCALE = 1.0 / 8.0   # 1/sqrt(HD)

NT = T // P         # 6
NO = CH // P        # 4
ND = D // P         # 4
NHID = HIDDEN // P  # 16

FFN_WSCALE = 2048.0   # FFN weights fp8 scale
QKV_WSCALE = 64.0     # qkv/out-proj weights fp8 scale

TILE_MAP = [1, 2, 4, 5, 0, 3]   # storage slot -> original tile
OWN_TILE = [0, 1, 2, 3]         # storage slot holding out-proj block tt

# scores-tile column offsets: [kt0 s0|s1, kt1 s0|s1, kt2 s0|s1]
def _scol(kt, stp):
    return (stp * 128, 256 + stp * 256, 768 + stp * 128)[kt]

_nc = None
LAST_EXEC_NS = None
LAST_RESULTS = None


def _body(ctx, tc, I, y):
    import concourse.bass as bass  # noqa: F401
    from concourse import mybir
    from concourse.masks import make_identity

    nc = tc.nc
    f32 = mybir.dt.float32
    bf16 = mybir.dt.bfloat16
    fp8 = mybir.dt.float8e4
    AF = mybir.ActivationFunctionType
    OP = mybir.AluOpType
    PM = mybir.MatmulPerfMode

    consts = ctx.enter_context(tc.tile_pool(name="consts", bufs=1))
    big = ctx.enter_context(tc.tile_pool(name="big", bufs=1))
    work = ctx.enter_context(tc.tile_pool(name="work", bufs=6))
    pexp = ctx.enter_context(tc.tile_pool(name="pexp", bufs=4))
    pps = ctx.enter_context(tc.tile_pool(name="pps", bufs=4, space="PSUM"))
    psc = ctx.enter_context(tc.tile_pool(name="psc", bufs=2, space="PSUM"))

    mm = nc.tensor.matmul
    RS = 1.0 / QKV_WSCALE

    def bcast(ap, p=P):
        return bass.AP(tensor=ap.tensor, offset=ap.offset,
                       ap=[[0, p]] + [list(d) for d in ap.ap])

    def frep(ap, n):
        """repeat a [1, F] AP n times along a new outer free dim."""
        return bass.AP(tensor=ap.tensor, offset=ap.offset,
                       ap=[list(ap.ap[0]), [0, n], list(ap.ap[1])])

    # ---------- constants / table prefetch / warmup ----------
    junk_in = consts.tile([P, CH], bf16, tag="junk")
    nc.vector.memset(junk_in, 0.25)
    epst = consts.tile([P, 1], f32, tag="eps")
    nc.vector.memset(epst, EPS)
    # sqrt table set loads now, during setup/DMA
    dum = consts.tile([1, 2], f32, tag="dum")
    nc.scalar.activation(dum[:, 0:1], epst[0:1, 0:1], AF.Sqrt)

    ident = consts.tile([P, P], bf16, tag="ident")
    make_identity(nc, ident)
    onesd = consts.tile([P, P], bf16, tag="onesd")
    nc.vector.memset(onesd, 1.0)
    negd = consts.tile([1, P], bf16, tag="negd")
    nc.vector.memset(negd, -1.0)
    idents = consts.tile([P, P], bf16, tag="idents")
    nc.vector.tensor_scalar_mul(idents, ident, QKV_WSCALE)

    for _ in range(12):
        jp = pps.tile([P, CH], f32, tag="ps")
        mm(jp, junk_in[:, 0:P], junk_in, start=True, stop=True)

    # ---------- input DMAs (one queue; priority order, no gates) ----------
    x_sb = big.tile([P, NT, D], bf16, tag="x")
    for j in range(NT):
        nc.sync.dma_start(out=x_sb[:, j, :], in_=I["xc"][:, j, :])
    bcons = consts.tile([P, 24], f32, tag="bcons")
    nc.sync.dma_start(out=bcons, in_=I["bcons"])
    bq_sb = bcons[:, 0:4]
    bk_sb = bcons[:, 4:8]
    b1_sb = bcons[:, 8:24]
    wk_sb = big.tile([P, ND, D], fp8, tag="wk")
    nc.sync.dma_start(out=wk_sb, in_=I["wkT"])
    masks_sb = consts.tile([P, 1024], bf16, tag="masks")
    nc.sync.dma_start(out=masks_sb, in_=I["masks"])
    wq_sb = big.tile([P, ND, D], fp8, tag="wq")
    nc.sync.dma_start(out=wq_sb, in_=I["wqT"])
    wv_sb = big.tile([P, ND, D], fp8, tag="wv")
    nc.sync.dma_start(out=wv_sb, in_=I["wvT"])
    wo_sb = big.tile([P, ND, D], fp8, tag="wo")
    nc.sync.dma_start(out=wo_sb, in_=I["woT"])
    w1_sb = big.tile([P, ND, HIDDEN], fp8, tag="w1")
    nc.sync.dma_start(out=w1_sb, in_=I["w1T"])
    w2_sb = big.tile([P, NHID, D], fp8, tag="w2")
    nc.sync.dma_start(out=w2_sb, in_=I["w2T"])
    b2s_sb = consts.tile([1, D], bf16, tag="b2s")
    nc.sync.dma_start(out=b2s_sb, in_=I["b2s"])
    bo_sb = consts.tile([P, D], f32, tag="bo")
    nc.gpsimd.dma_start(out=bo_sb, in_=bcast(I["bo"]))

    # ---------- LN1 stats + fused normalize-transpose ----------
    xT = big.tile([P, ND, T], fp8, tag="xT")
    xTr = xT.rearrange("p d (s c) -> p d s c", s=2)   # [P, ND, 2, 384]
    dg = big.tile([P, NT, P], bf16, tag="dg")

    lastr = [None]

    def emit_ln_stats(j, nmr2, src, dgt, scl, hi=False):
        from contextlib import nullcontext
        st = work.tile([P, 6], f32, tag="bnst")
        nc.vector.bn_stats(st, src)
        with (tc.high_priority() if hi else nullcontext()):
            mv = work.tile([P, 2], f32, tag="bnmv")
            nc.vector.bn_aggr(mv, st)
            r = work.tile([P, 1], f32, tag="lnr")
            nc.scalar.activation(r, mv[:, 1:2], AF.Sqrt, bias=epst, scale=1.0)
            lastr[0] = r
            r2 = work.tile([P, 1], f32, tag="lnr2")
            nc.vector.reciprocal(r2, r)
            nc.scalar.activation(nmr2[:, j % 2:j % 2 + 1], mv[:, 0:1],
                                 AF.Identity, scale=r2)
            # diag(rstd) for the fused normalize-transpose
            nc.vector.tensor_scalar_mul(dgt, scl, r2)

    def emit_nmrow(nmr2):
        # mu*rstd columns as one [1, 256] row at base partition 0
        pnm = psc.tile([1, 2 * P], bf16, tag="sc")
        for jj in range(2):
            nc.tensor.transpose(pnm[:, jj * P:(jj + 1) * P],
                                nmr2[:, jj:jj + 1], ident)
        nmrow = work.tile([1, 2 * P], bf16, tag="nmrow")
        nc.vector.tensor_copy(nmrow, pnm)
        return nmrow

    def emit_xpose(j0, nmr2, src3, dgt3, dsts):
        """dsts: list of (dest AP, engine) for the dt pairs (0,1) and (2,3);
        each dest is [P, 2, 256] (dt-pair x (tile j0 | tile j0+1))."""
        nmrow = emit_nmrow(nmr2)
        for half, (dst, eng) in enumerate(dsts):
            pt = pps.tile([P, 4 * P], f32, tag="ps")
            # only the first mm in the bank may use start=True (it clears the
            # whole bank's has_written bits); later regions write fresh with
            # start=False, and one batched rank-1 closes the bank.
            for di in range(2):
                dt_ = 2 * half + di
                for jj in range(2):
                    reg = pt[:, di * 2 * P + jj * P:di * 2 * P + (jj + 1) * P]
                    mm(reg, src3[:, j0 + jj, dt_ * P:(dt_ + 1) * P],
                       dgt3[:, j0 + jj, :],
                       start=(di == 0 and jj == 0), stop=False)
            mm(pt, negd, frep(nmrow, 2), start=False, stop=True)
            eng(dst, pt.rearrange("p (d c) -> p d c", d=2))

    for i, j0 in enumerate((0, 2, 4)):
        nmr2 = work.tile([P, 2], bf16, tag="nmr2")
        emit_ln_stats(j0, nmr2, x_sb[:, j0, :], dg[:, j0, :], ident, hi=True)
        if j0 == 0:
            for _ in range(3):   # warm-up bridge anchored on tile0's diag
                jp = pps.tile([P, CH], f32, tag="ps")
                mm(jp, dg[:, 0, :], junk_in, start=True, stop=True)
        emit_ln_stats(j0 + 1, nmr2, x_sb[:, j0 + 1, :], dg[:, j0 + 1, :],
                      ident, hi=True)
        if j0 == 0:
            for _ in range(3):   # bridge anchored on tile1's diag
                jp = pps.tile([P, CH], f32, tag="ps")
                mm(jp, dg[:, 1, :], junk_in, start=True, stop=True)
        o0 = TILE_MAP[j0]
        if j0 == 4:
            # halo tiles: columns 0:128 of each stream; per-dt destinations
            dsts = [
                (xTr[:, 0:2, :, 0:P], nc.scalar.copy),
                (xTr[:, 2:4, :, 0:P], nc.vector.tensor_copy),
            ]
        elif j0 == 0:
            dsts = [
                (xT[:, 0:2, o0 * P:(o0 + 2) * P], nc.scalar.copy),
                (xT[:, 2:4, o0 * P:(o0 + 2) * P], nc.scalar.copy),
            ]
        else:
            dsts = [
                (xT[:, 0:2, o0 * P:(o0 + 2) * P], nc.vector.tensor_copy),
                (xT[:, 2:4, o0 * P:(o0 + 2) * P], nc.vector.tensor_copy),
            ]
        emit_xpose(j0, nmr2, x_sb, dg, dsts)
    # exp table set prefetch (data-dependent: loads during QKV matmuls)
    nc.scalar.activation(dum[:, 1:2], lastr[0][0:1, 0:1], AF.Exp)

    # ---------- QKV per head-pair + V blocks (fp8 DoubleRow) ----------
    qT = big.tile([P, 4, CH], bf16, tag="qT")
    kT = big.tile([P, 4, T], bf16, tag="kT")
    kTr = kT.rearrange("p o (s c) -> p o s c", s=2)
    v_sb = big.tile([P, 6, H, 65], bf16, tag="v")
    for i in range(6):
        nc.vector.memset(v_sb[:, i, :, 64:65], 1.0)

    def emit_k(ot):
        ps = pps.tile([P, CH], f32, tag="ps")
        for dp in (0, 2):
            for stp in range(2):
                mm(ps[:, stp * SQ:(stp + 1) * SQ],
                   wk_sb[:, dp:dp + 2, ot * P:(ot + 1) * P],
                   xTr[:, dp:dp + 2, stp, P:SL],
                   start=(dp == 0), stop=(dp == 2), perf_mode=PM.DoubleRow)
        nc.scalar.activation(kTr[:, ot, :, P:SL],
                             ps.rearrange("p (s c) -> p s c", s=2),
                             AF.Identity,
                             bias=bk_sb[:, ot:ot + 1], scale=RS)
        ph = pps.tile([P, 256], f32, tag="ps")
        for dp in (0, 2):
            for stp in range(2):
                mm(ph[:, stp * P:(stp + 1) * P],
                   wk_sb[:, dp:dp + 2, ot * P:(ot + 1) * P],
                   xTr[:, dp:dp + 2, stp, 0:P],
                   start=(dp == 0), stop=(dp == 2), perf_mode=PM.DoubleRow)
        nc.scalar.activation(kTr[:, ot, :, 0:P],
                             ph.rearrange("p (s c) -> p s c", s=2),
                             AF.Identity,
                             bias=bk_sb[:, ot:ot + 1], scale=RS)

    def emit_q(ot):
        ps = pps.tile([P, CH], f32, tag="ps")
        for dp in (0, 2):
            for stp in range(2):
                mm(ps[:, stp * SQ:(stp + 1) * SQ],
                   wq_sb[:, dp:dp + 2, ot * P:(ot + 1) * P],
                   xTr[:, dp:dp + 2, stp, P:SL],
                   start=(dp == 0), stop=(dp == 2), perf_mode=PM.DoubleRow)
        nc.vector.tensor_scalar_mul(qT[:, ot, :], ps, RS)

    def emit_v_block(stp, i):
        ps = pps.tile([P, D], f32, tag="ps")
        c0 = SL * stp + P * i
        for dp in (0, 2):
            mm(ps, xT[:, dp:dp + 2, c0:c0 + P], wv_sb[:, dp:dp + 2, :],
               start=(dp == 0), stop=(dp == 2), perf_mode=PM.DoubleRow)
        nc.vector.tensor_scalar_mul(
            v_sb[:, stp * 3 + i, :, 0:64],
            ps.rearrange("p (h c) -> p h c", h=H), RS)

    # ---------- attention ----------
    oU = big.tile([P, 8, CH], bf16, tag="oU")
    oT = big.tile([P, 4, CH], fp8, tag="oT")

    def emit_S(ci, hp, hh):
        lo = hh * 64
        sc = psc.tile([P, 1024], f32, tag="sc")
        for kt in range(3):
            qw = SQ if kt == 1 else P
            for stp in range(2):
                q0 = stp * SQ + (0 if kt < 2 else P)
                mm(sc[:, _scol(kt, stp):_scol(kt, stp) + qw],
                   kT[lo:lo + 64, hp, SL * stp + P * kt:SL * stp + P * kt + P],
                   qT[lo:lo + 64, hp, q0:q0 + qw],
                   start=True, stop=True)
        p_raw = pexp.tile([P, 1024], bf16, tag="p_raw")
        nc.scalar.activation(p_raw, sc, AF.Exp, scale=SCALE)
        p_sb = pexp.tile([P, 1024], bf16, tag="p_sb")
        nc.vector.tensor_mul(p_sb, p_raw, masks_sb)
        return p_raw, p_sb

    def emit_PV(ci, hp, hh, p_sb):
        h = 2 * hp + hh
        po = pps.tile([P, CH], f32, tag="ps")
        ncol = 65 if hh == 0 else 64
        base = 0 if hh == 0 else 64
        for stp in range(2):
            qa = stp * SQ
            regions = (
                (qa, (_scol(0, stp), 0), (_scol(1, stp), 1)),
                (qa + P, (_scol(1, stp) + P, 1), (_scol(2, stp), 2)),
            )
            for q_out, (cA, iA), (cB, iB) in regions:
                mm(po[base:base + ncol, q_out:q_out + P],
                   v_sb[:, stp * 3 + iA, h, 0:ncol],
                   p_sb[:, cA:cA + P], start=True, stop=False)
                mm(po[base:base + ncol, q_out:q_out + P],
                   v_sb[:, stp * 3 + iB, h, 0:ncol],
                   p_sb[:, cB:cB + P], start=False, stop=True)
                if hh == 1:  # denominator, replicated into rows 0:64
                    mm(po[0:64, q_out:q_out + P], onesd[:, 0:64],
                       p_sb[:, cA:cA + P], start=True, stop=False)
                    mm(po[0:64, q_out:q_out + P], onesd[:, 0:64],
                       p_sb[:, cB:cB + P], start=False, stop=True)
        span = 65 if hh == 0 else P
        if ci in (1, 3, 7):
            nc.scalar.copy(oU[0:span, ci, :], po[0:span, :])
        else:
            nc.vector.tensor_copy(oU[0:span, ci, :], po[0:span, :])

    def emit_norm(hp):
        rb_ps = pps.tile([P, CH], f32, tag="ps")
        mm(rb_ps[0:64, :], onesd[64:65, 0:64], oU[64:65, 2 * hp, :],
           start=True, stop=True)
        mm(rb_ps[64:128, :], onesd[0:1, 0:64], oU[0:1, 2 * hp + 1, :],
           start=True, stop=True)
        rb = work.tile([P, CH], f32, tag="rb")
        nc.vector.reciprocal_approx_fast(out=rb, in_=rb_ps)
        rbh = work.tile([P, CH], bf16, tag="rbh")
        nc.vector.tensor_copy(rbh, rb)
        for hh in range(2):
            lo = hh * 64
            eng = nc.vector if hp == 3 else nc.gpsimd
            eng.tensor_mul(oT[lo:lo + 64, hp, :],
                           oU[lo:lo + 64, 2 * hp + hh, :],
                           rbh[lo:lo + 64, :])

    # interleaved QKV + attention chain schedule
    emit_k(0)
    emit_q(0)
    _, p0 = emit_S(0, 0, 0)
    emit_k(1)
    emit_q(1)
    _, p1 = emit_S(1, 0, 1)
    for i in (1, 2):
        for stp in range(2):
            emit_v_block(stp, i)
    for stp in range(2):
        emit_v_block(stp, 0)
    # pre-add the out-proj bias into the residual source (gpsimd slack)
    for tt in range(NO):
        nc.gpsimd.tensor_add(x_sb[:, OWN_TILE[tt], :],
                             x_sb[:, OWN_TILE[tt], :], bo_sb)
    emit_PV(0, 0, 0, p0)
    emit_k(2)
    emit_q(2)
    _, p2 = emit_S(2, 1, 0)
    emit_PV(1, 0, 1, p1)
    emit_norm(0)
    emit_k(3)
    emit_q(3)
    _, p3 = emit_S(3, 1, 1)
    emit_PV(2, 1, 0, p2)
    _, p4 = emit_S(4, 2, 0)
    emit_PV(3, 1, 1, p3)
    emit_norm(1)
    _, p5 = emit_S(5, 2, 1)
    emit_PV(4, 2, 0, p4)
    _, p6 = emit_S(6, 3, 0)
    emit_PV(5, 2, 1, p5)
    emit_norm(2)
    praw7, p7 = emit_S(7, 3, 1)
    # sqrt table set prefetch for LN2 (loads during PV6/PV7/out-proj)
    nc.scalar.activation(dum[:, 0:1], praw7[0:1, 0:1], AF.Sqrt)
    emit_PV(6, 3, 0, p6)
    emit_PV(7, 3, 1, p7)
    emit_norm(3)
    # keep the PE warm across the norm3 -> out-proj handoff
    for _ in range(8):
        jp = psc.tile([P, 1024], f32, tag="sc")
        mm(jp[:, 0:256], junk_in[:, 0:P], junk_in[:, 0:256],
           start=True, stop=True)

    # ---------- out projection + residual + LN2 + fused x2T, per tile ----
    res1 = big.tile([P, NO, D], bf16, tag="res1")
    x2T = big.tile([P, ND, CH], fp8, tag="x2T")
    dg2 = big.tile([P, NO, P], bf16, tag="dg2")

    def emit_outproj(tt, nmr2):
        ps = pps.tile([P, D], f32, tag="ps")
        for dp in (0, 2):
            mm(ps, oT[:, dp:dp + 2, tt * P:(tt + 1) * P],
               wo_sb[:, dp:dp + 2, :],
               start=(dp == 0), stop=False, perf_mode=PM.DoubleRow)
        mm(ps, idents, x_sb[:, OWN_TILE[tt], :], start=False, stop=True)
        if tt % 2 == 0:
            nc.scalar.activation(res1[:, tt, :], ps, AF.Identity, scale=RS)
        else:
            nc.vector.tensor_scalar_mul(res1[:, tt, :], ps, RS)
        emit_ln_stats(tt, nmr2, res1[:, tt, :], dg2[:, tt, :], ident)

    nmr2a = work.tile([P, 2], bf16, tag="nmr2")
    nmr2b = work.tile([P, 2], bf16, tag="nmr2")
    emit_outproj(0, nmr2a)
    emit_outproj(1, nmr2a)
    emit_outproj(2, nmr2b)
    emit_outproj(3, nmr2b)
    # gelu table prefetch (data-dependent: loads during x2T)
    nc.scalar.activation(dum[:, 1:2], lastr[0][0:1, 0:1], AF.Gelu)
    # keep the PE warm across the LN2 -> FFN handoff
    for _ in range(9):
        jp = pps.tile([P, CH], f32, tag="ps")
        mm(jp, junk_in[:, 0:P], junk_in, start=True, stop=True)
    emit_xpose(0, nmr2a, res1, dg2, [
        (x2T[:, 0:2, 0:2 * P], nc.vector.tensor_copy),
        (x2T[:, 2:4, 0:2 * P], nc.vector.tensor_copy),
    ])
    emit_xpose(2, nmr2b, res1, dg2, [
        (x2T[:, 0:2, 2 * P:4 * P], nc.vector.tensor_copy),
        (x2T[:, 2:4, 2 * P:4 * P], nc.scalar.copy),
    ])

    # ---------- FFN, interleaved per ht-pair; fp8 DoubleRow ----------
    g_sb = big.tile([P, NHID, CH], fp8, tag="g")
    fp0 = psc.tile([P, 1024], f32, tag="sc")
    fp1 = psc.tile([P, 1024], f32, tag="sc")
    fp2 = pps.tile([P, D], f32, tag="ps")
    fp3 = pps.tile([P, D], f32, tag="ps")
    fview = [fp0[:, 0:D], fp1[:, 0:D], fp2, fp3]

    def emit_ffn2(pr):
        for tt in range(NO):
            mm(fview[tt], g_sb[:, 2 * pr:2 * pr + 2, tt * P:(tt + 1) * P],
               w2_sb[:, 2 * pr:2 * pr + 2, :],
               start=(pr == 0), stop=False, perf_mode=PM.DoubleRow)

    for pr in range(NHID // 2):
        for ht in (2 * pr, 2 * pr + 1):
            ps = pps.tile([P, CH], f32, tag="ps")
            for dt_ in (0, 2):
                mm(ps, w1_sb[:, dt_:dt_ + 2, ht * P:(ht + 1) * P],
                   x2T[:, dt_:dt_ + 2, :],
                   start=(dt_ == 0), stop=(dt_ == 2), perf_mode=PM.DoubleRow)
            nc.scalar.activation(g_sb[:, ht, :], ps, AF.Gelu,
                                 bias=b1_sb[:, ht:ht + 1],
                                 scale=1.0 / FFN_WSCALE)
        if pr > 0:
            emit_ffn2(pr - 1)   # one pair behind: gelu latency stays hidden
    emit_ffn2(NHID // 2 - 1)

    fin = big.tile([P, NO, D], bf16, tag="fin")
    yr = y.rearrange("(j p) d -> p j d", p=P)
    for tt in range(NO):
        # b2 (pre-scaled by FFN_WSCALE) enters via a rank-1 matmul that also
        # closes the accumulation group
        mm(fview[tt], onesd[0:1, :], b2s_sb, start=False, stop=True)
        nc.vector.scalar_tensor_tensor(
            out=fin[:, tt, :], in0=fview[tt], scalar=1.0 / FFN_WSCALE,
            in1=res1[:, tt, :], op0=OP.mult, op1=OP.add)
        nc.sync.dma_start(out=yr[:, tt, :], in_=fin[:, tt, :])


def _build():
    from contextlib import ExitStack

    import concourse.bacc as bacc
    import concourse.tile as tile
    from concourse import mybir

    f32 = mybir.dt.float32
    bf16 = mybir.dt.bfloat16
    fp8 = mybir.dt.float8e4
    nc = bacc.Bacc("TRN2", target_bir_lowering=False, debug=False,
                   enable_asserts=False, num_devices=NCORES)
    I = {}

    def inp(name, shape, dt_):
        I[name] = nc.dram_tensor(name, list(shape), dt_, kind="ExternalInput").ap()

    inp("xc", (P, NT, D), bf16)
    inp("wqT", (P, ND, D), fp8)
    inp("wkT", (P, ND, D), fp8)
    inp("wvT", (P, ND, D), fp8)
    inp("bcons", (P, 24), f32)
    inp("woT", (P, ND, D), fp8)
    inp("bo", (D,), f32)
    inp("w1T", (P, ND, HIDDEN), fp8)
    inp("w2T", (P, NHID, D), fp8)
    inp("b2s", (1, D), bf16)
    inp("masks", (P, 1024), bf16)
    y = nc.dram_tensor("y", [CH, D], bf16, kind="ExternalOutput").ap()

    with tile.TileContext(nc) as tc:
        with ExitStack() as ctx:
            _body(ctx, tc, I, y)
    nc.compile()
    return nc


def _host_masks():
    import ml_dtypes
    sk = np.arange(SL)[:, None]
    sq = np.arange(SL - SQ, SL)[None, :]
    valid = ((sq - sk >= 0) & (sq - sk <= SW)).astype(np.float32)  # [384, 256]
    kt0 = valid[0:P, 0:P]           # keys 0:128, queries 0:128
    kt1 = valid[P:2 * P, :]         # keys 128:256, all queries
    kt2 = valid[2 * P:3 * P, P:SQ]  # keys 256:384, queries 128:256
    m = np.concatenate([kt0, kt0, kt1, kt1, kt2, kt2], axis=1)  # [128, 1024]
    m = m.astype(ml_dtypes.bfloat16)
    m0 = m.copy()
    m0[:, 0:256] = 0.0  # first chunk of each batch: halo keys invalid
    return np.ascontiguousarray(m), np.ascontiguousarray(m0)


def get_nc():
    global _nc
    if _nc is None:
        _nc = _build()
    return _nc


def _pmaj(a, p=P):
    """[N*p, F...] row-major -> [p, N, F...] partition-major contiguous."""
    n = a.shape[0] // p
    return np.ascontiguousarray(
        a.reshape((n, p) + a.shape[1:]).transpose((1, 0) + tuple(range(2, a.ndim + 1))))


def make_in_maps(inputs):
    import ml_dtypes
    f = np.float32
    bf = ml_dtypes.bfloat16
    fp8 = ml_dtypes.float8_e4m3

    def to8(a):
        return np.clip(a * QKV_WSCALE, -240, 240).astype(fp8)

    x = np.asarray(inputs["x"], f)
    qkv_w = np.asarray(inputs["qkv_w"], f)
    n1w = np.asarray(inputs["norm1_w"], f)
    n1b = np.asarray(inputs["norm1_b"], f)
    wqkv_f = qkv_w * n1w[None, :]
    bqkv = qkv_w @ n1b + np.asarray(inputs["qkv_b"], f)
    wT = np.ascontiguousarray(wqkv_f.T)        # [D, 3D]
    wqT = _pmaj(to8(wT[:, 0:D].copy()))
    wkT = _pmaj(to8(wT[:, D:2 * D].copy()))
    wvT = _pmaj(to8(wT[:, 2 * D:3 * D].copy()))
    bq = np.ascontiguousarray(bqkv[0:D].reshape(4, P).T)
    bk = np.ascontiguousarray(bqkv[D:2 * D].reshape(4, P).T)
    bv = bqkv[2 * D:3 * D]

    out_w = np.asarray(inputs["out_w"], f)
    woT = _pmaj(to8(np.ascontiguousarray(out_w.T)))
    # fold the V bias through the out projection (softmax weights sum to 1)
    bo = np.ascontiguousarray(np.asarray(inputs["out_b"], f) + out_w @ bv)

    w1 = np.asarray(inputs["ffn_w1"], f)
    n2w = np.asarray(inputs["norm2_w"], f)
    n2b = np.asarray(inputs["norm2_b"], f)
    w1T = _pmaj(np.clip(np.ascontiguousarray((w1 * n2w[None, :]).T)
                        * FFN_WSCALE, -240, 240).astype(fp8))
    b1v = w1 @ n2b + np.asarray(inputs["ffn_b1"], f)
    b1 = np.ascontiguousarray(b1v.reshape(NHID, P).T)
    w2T = _pmaj(np.clip(np.ascontiguousarray(np.asarray(inputs["ffn_w2"], f).T)
                        * FFN_WSCALE, -240, 240).astype(fp8))
    b2s = np.ascontiguousarray(
        (np.asarray(inputs["ffn_b2"], f) * FFN_WSCALE).reshape(1, D).astype(bf))

    bcons = np.ascontiguousarray(np.concatenate([bq, bk, b1], axis=1))
    masks, masks0 = _host_masks()
    shared = dict(wqT=wqT, wkT=wkT, wvT=wvT, bcons=bcons, woT=woT, bo=bo,
                  w1T=w1T, w2T=w2T, b2s=b2s)
    # stream-major permutation of the 768 halo+own tokens
    perm = np.concatenate([np.arange(0, T, 2), np.arange(1, T, 2)])
    in_maps = []
    for c in range(NCORES):
        b_, i = divmod(c, 4)
        own = x[b_, i * CH:(i + 1) * CH]
        if i == 0:
            # masked out anyway; real-ish values keep the halo LN variance
            # sane so rstd fits in fp8
            halo = x[b_, 0:HALO]
        else:
            halo = x[b_, i * CH - HALO:i * CH]
        xc = np.concatenate([halo, own], 0)[perm]
        xc = _pmaj(xc.astype(bf))
        xc = np.ascontiguousarray(xc[:, TILE_MAP, :])
        in_maps.append(dict(xc=xc, masks=(masks if i > 0 else masks0), **shared))
    return in_maps


def kernel(**inputs):
    global LAST_EXEC_NS, LAST_RESULTS
    from concourse.bass_utils import run_bass_kernel_spmd

    nc = get_nc()
    in_maps = make_in_maps(inputs)
    trace = bool(int(os.environ.get("BASS_KERNEL_TRACE", "0")))
    res = run_bass_kernel_spmd(nc, in_maps, core_ids=list(range(NCORES)),
                               trace=trace)
    LAST_EXEC_NS = res.exec_time_ns
    LAST_RESULTS = res
    out = np.zeros((B, L, D), np.float32)
    # kernel y rows are stream-major own tokens: un-permute
    operm = np.concatenate([np.arange(0, CH, 2), np.arange(1, CH, 2)])
    for c, r in enumerate(res.results):
        b_, i = divmod(c, 4)
        out[b_, i * CH + operm] = np.asarray(r["y"], np.float32)
    return out


# revision 37
# speedup vs baseline: 1.0569x; 1.0569x over previous
"""Trainium2 Bass kernel for a pre-norm transformer block with dilated
windowed causal attention (B=2, L=2048, D=512, H=8, DIL=2, WIN=256,
HIDDEN=2048).

Sharding: 8 cores = batch(2) x sequence-chunk(4 x 512 tokens). Each core
receives its 512-token chunk plus a 256-token halo (keys/values only) and
computes the full block for its tokens; no collectives.

v6 notes (on top of v3..v5):
  All projection weights ship fp8 (qkv/out scaled x64 host-side, FFN
  x2048); xT/oT are fp8 so QKV, V, out-proj and FFN all run DoubleRow
  fp8 matmuls at 2x.  Both layer-norm applies are folded into the
  transpose matmuls (diag(rstd) moving operand + one batched rank-1 mean
  mm per psum bank).  LN2 stats read the out-proj PSUM directly.  QKV is
  emitted per-head-pair interleaved with attention chains; junk-matmul
  fillers bridge the two PE gaps so the HAM clock gate stays warm.  ACT
  table prefetch dummies are data-dependent so the scheduler cannot
  hoist them.
"""
import os
import sys

os.environ.setdefault("MYCRO_LOCAL_CACHE", "1")
if "/opt/trn_rl_repo" not in sys.path:
    sys.path.insert(0, "/opt/trn_rl_repo")

import numpy as np

B, L, D, H, HD = 2, 2048, 512, 8, 64
HIDDEN = 4 * D
P = 128
CH = 512            # own tokens per core
HALO = 256
T = CH + HALO       # 768
NCORES = 8
EPS = 1e-5
SL = T // 2         # 384 keys per parity stream
SQ = CH // 2        # 256 queries per parity stream
SW = 128            # causal window in stream coords
SCALE = 1.0 / 8.0   # 1/sqrt(HD)

NT = T // P         # 6
NO = CH // P        # 4
ND = D // P         # 4
NHID = HIDDEN // P  # 16

FFN_WSCALE = 2048.0   # FFN weights fp8 scale
QKV_WSCALE = 64.0     # qkv/out-proj weights fp8 scale

TILE_MAP = [1, 2, 4, 5, 0, 3]   # storage slot -> original tile
OWN_TILE = [0, 1, 2, 3]         # storage slot holding out-proj block tt

# scores-tile column offsets: [kt0 s0|s1, kt1 s0|s1, kt2 s0|s1]
def _scol(kt, stp):
    return (stp * 128, 256 + stp * 256, 768 + stp * 128)[kt]

_nc = None
LAST_EXEC_NS = None
LAST_RESULTS = None


def _body(ctx, tc, I, y):
    import concourse.bass as bass  # noqa: F401
    from concourse import mybir
    from concourse.masks import make_identity

    nc = tc.nc
    f32 = mybir.dt.float32
    bf16 = mybir.dt.bfloat16
    fp8 = mybir.dt.float8e4
    AF = mybir.ActivationFunctionType
    OP = mybir.AluOpType
    PM = mybir.MatmulPerfMode

    consts = ctx.enter_context(tc.tile_pool(name="consts", bufs=1))
    big = ctx.enter_context(tc.tile_pool(name="big", bufs=1))
    work = ctx.enter_context(tc.tile_pool(name="work", bufs=6))
    pexp = ctx.enter_context(tc.tile_pool(name="pexp", bufs=4))
    pps = ctx.enter_context(tc.tile_pool(name="pps", bufs=4, space="PSUM"))
    psc = ctx.enter_context(tc.tile_pool(name="psc", bufs=2, space="PSUM"))

    mm = nc.tensor.matmul
    RS = 1.0 / QKV_WSCALE

    def bcast(ap, p=P):
        return bass.AP(tensor=ap.tensor, offset=ap.offset,
                       ap=[[0, p]] + [list(d) for d in ap.ap])

    def frep(ap, n):
        """repeat a [1, F] AP n times along a new outer free dim."""
        return bass.AP(tensor=ap.tensor, offset=ap.offset,
                       ap=[list(ap.ap[0]), [0, n], list(ap.ap[1])])

    # ---------- constants / table prefetch / warmup ----------
    junk_in = consts.tile([P, CH], bf16, tag="junk")
    nc.vector.memset(junk_in, 0.25)
    epst = consts.tile([P, 1], f32, tag="eps")
    nc.vector.memset(epst, EPS)
    # sqrt table set loads now, during setup/DMA
    dum = consts.tile([1, 2], f32, tag="dum")
    nc.scalar.activation(dum[:, 0:1], epst[0:1, 0:1], AF.Sqrt)

    ident = consts.tile([P, P], bf16, tag="ident")
    make_identity(nc, ident)
    onesd = consts.tile([P, P], bf16, tag="onesd")
    nc.vector.memset(onesd, 1.0)
    negd = consts.tile([1, P], bf16, tag="negd")
    nc.vector.memset(negd, -1.0)
    idents = consts.tile([P, P], bf16, tag="idents")
    nc.vector.tensor_scalar_mul(idents, ident, QKV_WSCALE)

    for _ in range(12):
        jp = pps.tile([P, CH], f32, tag="ps")
        mm(jp, junk_in[:, 0:P], junk_in, start=True, stop=True)

    # ---------- input DMAs (one queue; priority order, no gates) ----------
    x_sb = big.tile([P, NT, D], bf16, tag="x")
    for j in range(NT):
        nc.sync.dma_start(out=x_sb[:, j, :], in_=I["xc"][:, j, :])
    bcons = consts.tile([P, 24], f32, tag="bcons")
    nc.sync.dma_start(out=bcons, in_=I["bcons"])
    bq_sb = bcons[:, 0:4]
    bk_sb = bcons[:, 4:8]
    b1_sb = bcons[:, 8:24]
    wk_sb = big.tile([P, ND, D], fp8, tag="wk")
    nc.sync.dma_start(out=wk_sb, in_=I["wkT"])
    masks_sb = consts.tile([P, 1024], bf16, tag="masks")
    nc.sync.dma_start(out=masks_sb, in_=I["masks"])
    wq_sb = big.tile([P, ND, D], fp8, tag="wq")
    nc.sync.dma_start(out=wq_sb, in_=I["wqT"])
    wv_sb = big.tile([P, ND, D], fp8, tag="wv")
    nc.sync.dma_start(out=wv_sb, in_=I["wvT"])
    wo_sb = big.tile([P, ND, D], fp8, tag="wo")
    nc.sync.dma_start(out=wo_sb, in_=I["woT"])
    w1_sb = big.tile([P, ND, HIDDEN], fp8, tag="w1")
    nc.sync.dma_start(out=w1_sb, in_=I["w1T"])
    w2_sb = big.tile([P, NHID, D], fp8, tag="w2")
    nc.sync.dma_start(out=w2_sb, in_=I["w2T"])
    b2s_sb = consts.tile([1, D], bf16, tag="b2s")
    nc.sync.dma_start(out=b2s_sb, in_=I["b2s"])
    bo_sb = consts.tile([P, D], f32, tag="bo")
    nc.gpsimd.dma_start(out=bo_sb, in_=bcast(I["bo"]))

    # ---------- LN1 stats + fused normalize-transpose ----------
    xT = big.tile([P, ND, T], fp8, tag="xT")
    xTr = xT.rearrange("p d (s c) -> p d s c", s=2)   # [P, ND, 2, 384]
    dg = big.tile([P, NT, P], bf16, tag="dg")

    lastr = [None]

    def emit_ln_stats(j, nmr2, src, dgt, scl, hi=False):
        from contextlib import nullcontext
        st = work.tile([P, 6], f32, tag="bnst")
        nc.vector.bn_stats(st, src)
        with (tc.high_priority() if hi else nullcontext()):
            mv = work.tile([P, 2], f32, tag="bnmv")
            nc.vector.bn_aggr(mv, st)
            r = work.tile([P, 1], f32, tag="lnr")
            nc.scalar.activation(r, mv[:, 1:2], AF.Sqrt, bias=epst, scale=1.0)
            lastr[0] = r
            r2 = work.tile([P, 1], f32, tag="lnr2")
            nc.vector.reciprocal(r2, r)
            nc.scalar.activation(nmr2[:, j % 2:j % 2 + 1], mv[:, 0:1],
                                 AF.Identity, scale=r2)
            # diag(rstd) for the fused normalize-transpose
            nc.vector.tensor_scalar_mul(dgt, scl, r2)

    def emit_nmrow(nmr2):
        # mu*rstd columns as one [1, 256] row at base partition 0
        pnm = psc.tile([1, 2 * P], bf16, tag="sc")
        for jj in range(2):
            nc.tensor.transpose(pnm[:, jj * P:(jj + 1) * P],
                                nmr2[:, jj:jj + 1], ident)
        nmrow = work.tile([1, 2 * P], bf16, tag="nmrow")
        nc.vector.tensor_copy(nmrow, pnm)
        return nmrow

    def emit_xpose(j0, nmr2, src3, dgt3, dsts):
        """dsts: list of (dest AP, engine) for the dt pairs (0,1) and (2,3);
        each dest is [P, 2, 256] (dt-pair x (tile j0 | tile j0+1))."""
        nmrow = emit_nmrow(nmr2)
        for half, (dst, eng) in enumerate(dsts):
            pt = pps.tile([P, 4 * P], f32, tag="ps")
            # only the first mm in the bank may use start=True (it clears the
            # whole bank's has_written bits); later regions write fresh with
            # start=False, and one batched rank-1 closes the bank.
            for di in range(2):
                dt_ = 2 * half + di
                for jj in range(2):
                    reg = pt[:, di * 2 * P + jj * P:di * 2 * P + (jj + 1) * P]
                    mm(reg, src3[:, j0 + jj, dt_ * P:(dt_ + 1) * P],
                       dgt3[:, j0 + jj, :],
                       start=(di == 0 and jj == 0), stop=False)
            mm(pt, negd, frep(nmrow, 2), start=False, stop=True)
            eng(dst, pt.rearrange("p (d c) -> p d c", d=2))

    for i, j0 in enumerate((0, 2, 4)):
        nmr2 = work.tile([P, 2], bf16, tag="nmr2")
        emit_ln_stats(j0, nmr2, x_sb[:, j0, :], dg[:, j0, :], ident, hi=True)
        if j0 == 0:
            for _ in range(3):   # warm-up bridge anchored on tile0's diag
                jp = pps.tile([P, CH], f32, tag="ps")
                mm(jp, dg[:, 0, :], junk_in, start=True, stop=True)
        emit_ln_stats(j0 + 1, nmr2, x_sb[:, j0 + 1, :], dg[:, j0 + 1, :],
                      ident, hi=True)
        if j0 == 0:
            for _ in range(3):   # bridge anchored on tile1's diag
                jp = pps.tile([P, CH], f32, tag="ps")
                mm(jp, dg[:, 1, :], junk_in, start=True, stop=True)
        o0 = TILE_MAP[j0]
        if j0 == 4:
            # halo tiles: columns 0:128 of each stream; per-dt destinations
            dsts = [
                (xTr[:, 0:2, :, 0:P], nc.vector.tensor_copy),
                (xTr[:, 2:4, :, 0:P], nc.vector.tensor_copy),
            ]
        else:
            dsts = [
                (xT[:, 0:2, o0 * P:(o0 + 2) * P], nc.vector.tensor_copy),
                (xT[:, 2:4, o0 * P:(o0 + 2) * P], nc.vector.tensor_copy),
            ]
        emit_xpose(j0, nmr2, x_sb, dg, dsts)
    # exp table set prefetch (data-dependent: loads during QKV matmuls)
    nc.scalar.activation(dum[:, 1:2], lastr[0][0:1, 0:1], AF.Exp)

    # ---------- QKV per head-pair + V blocks (fp8 DoubleRow) ----------
    qT = big.tile([P, 4, CH], bf16, tag="qT")
    kT = big.tile([P, 4, T], bf16, tag="kT")
    kTr = kT.rearrange("p o (s c) -> p o s c", s=2)
    v_sb = big.tile([P, 6, H, 65], bf16, tag="v")
    for i in range(6):
        nc.vector.memset(v_sb[:, i, :, 64:65], 1.0)

    def emit_k(ot):
        ps = pps.tile([P, CH], f32, tag="ps")
        for dp in (0, 2):
            for stp in range(2):
                mm(ps[:, stp * SQ:(stp + 1) * SQ],
                   wk_sb[:, dp:dp + 2, ot * P:(ot + 1) * P],
                   xTr[:, dp:dp + 2, stp, P:SL],
                   start=(dp == 0), stop=(dp == 2), perf_mode=PM.DoubleRow)
        nc.scalar.activation(kTr[:, ot, :, P:SL],
                             ps.rearrange("p (s c) -> p s c", s=2),
                             AF.Identity,
                             bias=bk_sb[:, ot:ot + 1], scale=RS)
        ph = pps.tile([P, 256], f32, tag="ps")
        for dp in (0, 2):
            for stp in range(2):
                mm(ph[:, stp * P:(stp + 1) * P],
                   wk_sb[:, dp:dp + 2, ot * P:(ot + 1) * P],
                   xTr[:, dp:dp + 2, stp, 0:P],
                   start=(dp == 0), stop=(dp == 2), perf_mode=PM.DoubleRow)
        nc.scalar.activation(kTr[:, ot, :, 0:P],
                             ph.rearrange("p (s c) -> p s c", s=2),
                             AF.Identity,
                             bias=bk_sb[:, ot:ot + 1], scale=RS)

    def emit_q(ot):
        ps = pps.tile([P, CH], f32, tag="ps")
        for dp in (0, 2):
            for stp in range(2):
                mm(ps[:, stp * SQ:(stp + 1) * SQ],
                   wq_sb[:, dp:dp + 2, ot * P:(ot + 1) * P],
                   xTr[:, dp:dp + 2, stp, P:SL],
                   start=(dp == 0), stop=(dp == 2), perf_mode=PM.DoubleRow)
        nc.vector.tensor_scalar_mul(qT[:, ot, :], ps, RS)

    def emit_v_block(stp, i):
        ps = pps.tile([P, D], f32, tag="ps")
        c0 = SL * stp + P * i
        for dp in (0, 2):
            mm(ps, xT[:, dp:dp + 2, c0:c0 + P], wv_sb[:, dp:dp + 2, :],
               start=(dp == 0), stop=(dp == 2), perf_mode=PM.DoubleRow)
        nc.vector.tensor_scalar_mul(
            v_sb[:, stp * 3 + i, :, 0:64],
            ps.rearrange("p (h c) -> p h c", h=H), RS)

    # ---------- attention ----------
    oU = big.tile([P, 8, CH], bf16, tag="oU")
    oT = big.tile([P, 4, CH], fp8, tag="oT")

    def emit_S(ci, hp, hh):
        lo = hh * 64
        sc = psc.tile([P, 1024], f32, tag="sc")
        for kt in range(3):
            qw = SQ if kt == 1 else P
            for stp in range(2):
                q0 = stp * SQ + (0 if kt < 2 else P)
                mm(sc[:, _scol(kt, stp):_scol(kt, stp) + qw],
                   kT[lo:lo + 64, hp, SL * stp + P * kt:SL * stp + P * kt + P],
                   qT[lo:lo + 64, hp, q0:q0 + qw],
                   start=True, stop=True)
        p_raw = pexp.tile([P, 1024], bf16, tag="p_raw")
        nc.scalar.activation(p_raw, sc, AF.Exp, scale=SCALE)
        p_sb = pexp.tile([P, 1024], bf16, tag="p_sb")
        nc.vector.tensor_mul(p_sb, p_raw, masks_sb)
        return p_raw, p_sb

    def emit_PV(ci, hp, hh, p_sb):
        h = 2 * hp + hh
        po = pps.tile([P, CH], f32, tag="ps")
        ncol = 65 if hh == 0 else 64
        base = 0 if hh == 0 else 64
        for stp in range(2):
            qa = stp * SQ
            regions = (
                (qa, (_scol(0, stp), 0), (_scol(1, stp), 1)),
                (qa + P, (_scol(1, stp) + P, 1), (_scol(2, stp), 2)),
            )
            for q_out, (cA, iA), (cB, iB) in regions:
                mm(po[base:base + ncol, q_out:q_out + P],
                   v_sb[:, stp * 3 + iA, h, 0:ncol],
                   p_sb[:, cA:cA + P], start=True, stop=False)
                mm(po[base:base + ncol, q_out:q_out + P],
                   v_sb[:, stp * 3 + iB, h, 0:ncol],
                   p_sb[:, cB:cB + P], start=False, stop=True)
                if hh == 1:  # denominator, replicated into rows 0:64
                    mm(po[0:64, q_out:q_out + P], onesd[:, 0:64],
                       p_sb[:, cA:cA + P], start=True, stop=False)
                    mm(po[0:64, q_out:q_out + P], onesd[:, 0:64],
                       p_sb[:, cB:cB + P], start=False, stop=True)
        span = 65 if hh == 0 else P
        if ci in (1, 3, 7):
            nc.scalar.copy(oU[0:span, ci, :], po[0:span, :])
        else:
            nc.vector.tensor_copy(oU[0:span, ci, :], po[0:span, :])

    def emit_norm(hp):
        rb_ps = pps.tile([P, CH], f32, tag="ps")
        mm(rb_ps[0:64, :], onesd[64:65, 0:64], oU[64:65, 2 * hp, :],
           start=True, stop=True)
        mm(rb_ps[64:128, :], onesd[0:1, 0:64], oU[0:1, 2 * hp + 1, :],
           start=True, stop=True)
        rb = work.tile([P, CH], f32, tag="rb")
        nc.vector.reciprocal_approx_fast(out=rb, in_=rb_ps)
        rbh = work.tile([P, CH], bf16, tag="rbh")
        nc.vector.tensor_copy(rbh, rb)
        for hh in range(2):
            lo = hh * 64
            eng = nc.vector if hp == 3 else nc.gpsimd
            eng.tensor_mul(oT[lo:lo + 64, hp, :],
                           oU[lo:lo + 64, 2 * hp + hh, :],
                           rbh[lo:lo + 64, :])

    # interleaved QKV + attention chain schedule
    emit_k(0)
    emit_q(0)
    _, p0 = emit_S(0, 0, 0)
    emit_k(1)
    emit_q(1)
    _, p1 = emit_S(1, 0, 1)
    for i in (1, 2):
        for stp in range(2):
            emit_v_block(stp, i)
    for stp in range(2):
        emit_v_block(stp, 0)
    # pre-add the out-proj bias into the residual source (gpsimd slack)
    for tt in range(NO):
        nc.gpsimd.tensor_add(x_sb[:, OWN_TILE[tt], :],
                             x_sb[:, OWN_TILE[tt], :], bo_sb)
    emit_PV(0, 0, 0, p0)
    emit_k(2)
    emit_q(2)
    _, p2 = emit_S(2, 1, 0)
    emit_PV(1, 0, 1, p1)
    emit_norm(0)
    emit_k(3)
    emit_q(3)
    _, p3 = emit_S(3, 1, 1)
    emit_PV(2, 1, 0, p2)
    _, p4 = emit_S(4, 2, 0)
    emit_PV(3, 1, 1, p3)
    emit_norm(1)
    _, p5 = emit_S(5, 2, 1)
    emit_PV(4, 2, 0, p4)
    _, p6 = emit_S(6, 3, 0)
    emit_PV(5, 2, 1, p5)
    emit_norm(2)
    praw7, p7 = emit_S(7, 3, 1)
    # sqrt table set prefetch for LN2 (loads during PV6/PV7/out-proj)
    nc.scalar.activation(dum[:, 0:1], praw7[0:1, 0:1], AF.Sqrt)
    emit_PV(6, 3, 0, p6)
    emit_PV(7, 3, 1, p7)
    emit_norm(3)
    # keep the PE warm across the norm3 -> out-proj handoff
    for _ in range(12):
        jp = psc.tile([P, 1024], f32, tag="sc")
        mm(jp[:, 0:CH], junk_in[:, 0:P], junk_in,
           start=True, stop=True)

    # ---------- out projection + residual + LN2 + fused x2T, per tile ----
    res1 = big.tile([P, NO, D], bf16, tag="res1")
    x2T = big.tile([P, ND, CH], fp8, tag="x2T")
    dg2 = big.tile([P, NO, P], bf16, tag="dg2")

    def emit_outproj(tt, nmr2):
        ps = pps.tile([P, D], f32, tag="ps")
        for dp in (0, 2):
            mm(ps, oT[:, dp:dp + 2, tt * P:(tt + 1) * P],
               wo_sb[:, dp:dp + 2, :],
               start=(dp == 0), stop=False, perf_mode=PM.DoubleRow)
        mm(ps, idents, x_sb[:, OWN_TILE[tt], :], start=False, stop=True)
        if tt % 2 == 0:
            nc.scalar.activation(res1[:, tt, :], ps, AF.Identity, scale=RS)
        else:
            nc.vector.tensor_scalar_mul(res1[:, tt, :], ps, RS)
        emit_ln_stats(tt, nmr2, res1[:, tt, :], dg2[:, tt, :], ident)

    nmr2a = work.tile([P, 2], bf16, tag="nmr2")
    nmr2b = work.tile([P, 2], bf16, tag="nmr2")
    emit_outproj(0, nmr2a)
    emit_outproj(1, nmr2a)
    emit_outproj(2, nmr2b)
    emit_outproj(3, nmr2b)
    # gelu table prefetch (data-dependent: loads during x2T)
    nc.scalar.activation(dum[:, 1:2], lastr[0][0:1, 0:1], AF.Gelu)
    # keep the PE warm across the LN2 -> FFN handoff
    for _ in range(9):
        jp = pps.tile([P, CH], f32, tag="ps")
        mm(jp, junk_in[:, 0:P], junk_in, start=True, stop=True)
    emit_xpose(0, nmr2a, res1, dg2, [
        (x2T[:, 0:2, 0:2 * P], nc.vector.tensor_copy),
        (x2T[:, 2:4, 0:2 * P], nc.vector.tensor_copy),
    ])
    for _ in range(4):
        jp = pps.tile([P, CH], f32, tag="ps")
        mm(jp, junk_in[:, 0:P], junk_in, start=True, stop=True)
    emit_xpose(2, nmr2b, res1, dg2, [
        (x2T[:, 0:2, 2 * P:4 * P], nc.vector.tensor_copy),
        (x2T[:, 2:4, 2 * P:4 * P], nc.scalar.copy),
    ])

    # ---------- FFN, interleaved per ht-pair; fp8 DoubleRow ----------
    g_sb = big.tile([P, NHID, CH], fp8, tag="g")
    fp0 = psc.tile([P, 1024], f32, tag="sc")
    fp1 = psc.tile([P, 1024], f32, tag="sc")
    fp2 = pps.tile([P, D], f32, tag="ps")
    fp3 = pps.tile([P, D], f32, tag="ps")
    fview = [fp0[:, 0:D], fp1[:, 0:D], fp2, fp3]

    def emit_ffn2(pr):
        for tt in range(NO):
            mm(fview[tt], g_sb[:, 2 * pr:2 * pr + 2, tt * P:(tt + 1) * P],
               w2_sb[:, 2 * pr:2 * pr + 2, :],
               start=(pr == 0), stop=False, perf_mode=PM.DoubleRow)

    for pr in range(NHID // 2):
        for ht in (2 * pr, 2 * pr + 1):
            ps = pps.tile([P, CH], f32, tag="ps")
            for dt_ in (0, 2):
                mm(ps, w1_sb[:, dt_:dt_ + 2, ht * P:(ht + 1) * P],
                   x2T[:, dt_:dt_ + 2, :],
                   start=(dt_ == 0), stop=(dt_ == 2), perf_mode=PM.DoubleRow)
            nc.scalar.activation(g_sb[:, ht, :], ps, AF.Gelu,
                                 bias=b1_sb[:, ht:ht + 1],
                                 scale=1.0 / FFN_WSCALE)
        if pr > 0:
            emit_ffn2(pr - 1)   # one pair behind: gelu latency stays hidden
    emit_ffn2(NHID // 2 - 1)

    fin = big.tile([P, NO, D], bf16, tag="fin")
    yr = y.rearrange("(j p) d -> p j d", p=P)
    for tt in range(NO):
        # b2 (pre-scaled by FFN_WSCALE) enters via a rank-1 matmul that also
        # closes the accumulation group
        mm(fview[tt], onesd[0:1, :], b2s_sb, start=False, stop=True)
        nc.vector.scalar_tensor_tensor(
            out=fin[:, tt, :], in0=fview[tt], scalar=1.0 / FFN_WSCALE,
            in1=res1[:, tt, :], op0=OP.mult, op1=OP.add)
        nc.sync.dma_start(out=yr[:, tt, :], in_=fin[:, tt, :])


def _build():
    from contextlib import ExitStack

    import concourse.bacc as bacc
    import concourse.tile as tile
    from concourse import mybir

    f32 = mybir.dt.float32
    bf16 = mybir.dt.bfloat16
    fp8 = mybir.dt.float8e4
    nc = bacc.Bacc("TRN2", target_bir_lowering=False, debug=False,
                   enable_asserts=False, num_devices=NCORES)
    I = {}

    def inp(name, shape, dt_):
        I[name] = nc.dram_tensor(name, list(shape), dt_, kind="ExternalInput").ap()

    inp("xc", (P, NT, D), bf16)
    inp("wqT", (P, ND, D), fp8)
    inp("wkT", (P, ND, D), fp8)
    inp("wvT", (P, ND, D), fp8)
    inp("bcons", (P, 24), f32)
    inp("woT", (P, ND, D), fp8)
    inp("bo", (D,), f32)
    inp("w1T", (P, ND, HIDDEN), fp8)
    inp("w2T", (P, NHID, D), fp8)
    inp("b2s", (1, D), bf16)
    inp("masks", (P, 1024), bf16)
    y = nc.dram_tensor("y", [CH, D], bf16, kind="ExternalOutput").ap()

    with tile.TileContext(nc) as tc:
        with ExitStack() as ctx:
            _body(ctx, tc, I, y)
    nc.compile()
    return nc


def _host_masks():
    import ml_dtypes
    sk = np.arange(SL)[:, None]
    sq = np.arange(SL - SQ, SL)[None, :]
    valid = ((sq - sk >= 0) & (sq - sk <= SW)).astype(np.float32)  # [384, 256]
    kt0 = valid[0:P, 0:P]           # keys 0:128, queries 0:128
    kt1 = valid[P:2 * P, :]         # keys 128:256, all queries
    kt2 = valid[2 * P:3 * P, P:SQ]  # keys 256:384, queries 128:256
    m = np.concatenate([kt0, kt0, kt1, kt1, kt2, kt2], axis=1)  # [128, 1024]
    m = m.astype(ml_dtypes.bfloat16)
    m0 = m.copy()
    m0[:, 0:256] = 0.0  # first chunk of each batch: halo keys invalid
    return np.ascontiguousarray(m), np.ascontiguousarray(m0)


def get_nc():
    global _nc
    if _nc is None:
        _nc = _build()
    return _nc


def _pmaj(a, p=P):
    """[N*p, F...] row-major -> [p, N, F...] partition-major contiguous."""
    n = a.shape[0] // p
    return np.ascontiguousarray(
        a.reshape((n, p) + a.shape[1:]).transpose((1, 0) + tuple(range(2, a.ndim + 1))))


def make_in_maps(inputs):
    import ml_dtypes
    f = np.float32
    bf = ml_dtypes.bfloat16
    fp8 = ml_dtypes.float8_e4m3

    def to8(a):
        return np.clip(a * QKV_WSCALE, -240, 240).astype(fp8)

    x = np.asarray(inputs["x"], f)
    qkv_w = np.asarray(inputs["qkv_w"], f)
    n1w = np.asarray(inputs["norm1_w"], f)
    n1b = np.asarray(inputs["norm1_b"], f)
    wqkv_f = qkv_w * n1w[None, :]
    bqkv = qkv_w @ n1b + np.asarray(inputs["qkv_b"], f)
    wT = np.ascontiguousarray(wqkv_f.T)        # [D, 3D]
    wqT = _pmaj(to8(wT[:, 0:D].copy()))
    wkT = _pmaj(to8(wT[:, D:2 * D].copy()))
    wvT = _pmaj(to8(wT[:, 2 * D:3 * D].copy()))
    bq = np.ascontiguousarray(bqkv[0:D].reshape(4, P).T)
    bk = np.ascontiguousarray(bqkv[D:2 * D].reshape(4, P).T)
    bv = bqkv[2 * D:3 * D]

    out_w = np.asarray(inputs["out_w"], f)
    woT = _pmaj(to8(np.ascontiguousarray(out_w.T)))
    # fold the V bias through the out projection (softmax weights sum to 1)
    bo = np.ascontiguousarray(np.asarray(inputs["out_b"], f) + out_w @ bv)

    w1 = np.asarray(inputs["ffn_w1"], f)
    n2w = np.asarray(inputs["norm2_w"], f)
    n2b = np.asarray(inputs["norm2_b"], f)
    w1T = _pmaj(np.clip(np.ascontiguousarray((w1 * n2w[None, :]).T)
                        * FFN_WSCALE, -240, 240).astype(fp8))
    b1v = w1 @ n2b + np.asarray(inputs["ffn_b1"], f)
    b1 = np.ascontiguousarray(b1v.reshape(NHID, P).T)
    w2T = _pmaj(np.clip(np.ascontiguousarray(np.asarray(inputs["ffn_w2"], f).T)
                        * FFN_WSCALE, -240, 240).astype(fp8))
    b2s = np.ascontiguousarray(
        (np.asarray(inputs["ffn_b2"], f) * FFN_WSCALE).reshape(1, D).astype(bf))

    bcons = np.ascontiguousarray(np.concatenate([bq, bk, b1], axis=1))
    masks, masks0 = _host_masks()
    shared = dict(wqT=wqT, wkT=wkT, wvT=wvT, bcons=bcons, woT=woT, bo=bo,
                  w1T=w1T, w2T=w2T, b2s=b2s)
    # stream-major permutation of the 768 halo+own tokens
    perm = np.concatenate([np.arange(0, T, 2), np.arange(1, T, 2)])
    in_maps = []
    for c in range(NCORES):
        b_, i = divmod(c, 4)
        own = x[b_, i * CH:(i + 1) * CH]
        if i == 0:
            # masked out anyway; real-ish values keep the halo LN variance
            # sane so rstd fits in fp8
            halo = x[b_, 0:HALO]
        else:
            halo = x[b_, i * CH - HALO:i * CH]
        xc = np.concatenate([halo, own], 0)[perm]
        xc = _pmaj(xc.astype(bf))
        xc = np.ascontiguousarray(xc[:, TILE_MAP, :])
        in_maps.append(dict(xc=xc, masks=(masks if i > 0 else masks0), **shared))
    return in_maps


def kernel(**inputs):
    global LAST_EXEC_NS, LAST_RESULTS
    from concourse.bass_utils import run_bass_kernel_spmd

    nc = get_nc()
    in_maps = make_in_maps(inputs)
    trace = bool(int(os.environ.get("BASS_KERNEL_TRACE", "0")))
    res = run_bass_kernel_spmd(nc, in_maps, core_ids=list(range(NCORES)),
                               trace=trace)
    LAST_EXEC_NS = res.exec_time_ns
    LAST_RESULTS = res
    out = np.zeros((B, L, D), np.float32)
    # kernel y rows are stream-major own tokens: un-permute
    operm = np.concatenate([np.arange(0, CH, 2), np.arange(1, CH, 2)])
    for c, r in enumerate(res.results):
        b_, i = divmod(c, 4)
        out[b_, i * CH + operm] = np.asarray(r["y"], np.float32)
    return out
